# revision 29
# baseline (speedup 1.0000x reference)
"""Trainium2 Bass kernel for nn_CNN2_P (dense CNN + MLP head).

Pure data-parallel over 8 NeuronCores: batch 2048 -> 256 per core, all
weights replicated. Host-side prep re-tiles weights into PE-friendly
layouts and casts to bf16; the device kernel runs conv1/2/3 as
accumulating matmuls (channels on partitions). conv3's drain writes the
resident l-major y3 through an [l, 4-contiguous-samples] view (8-byte
runs, cheap) while reading the sample-major PSUM strided, so fc1's rhs
slices stay fully contiguous (a strided matmul rhs runs ~4x slow on the
PE). The streamed fc1 weights flow through a 20-slot SBUF ring whose
head is prefetched during the conv phase; fc2 chains off the fc1 PSUM
drains per batch-half. Warm-up matmuls on a memset tile ramp the PE
DVFS p-state through the startup DMA latency, and DMA triggers are
placed so weight/chunk transfers self-pace behind the drain queues.
"""

import os

import numpy as np
import ml_dtypes

import concourse.mybir as mybir
import concourse.bacc as bacc
import concourse.tile as tile
from concourse.bass_utils import run_bass_kernel_spmd

# Problem constants (hardcoded per contract).
CL, IL = 128, 64          # context length, instruction length
CH = 256                  # channels in all three convs
L1, L2, L3 = 127, 125, 123
F1, OUT = 1024, 16
BATCH = 2048
NCORES = 8

BF16 = ml_dtypes.bfloat16

_CACHE = {}

WF1_RING = 18             # persistent SBUF ring depth for fc1 weights
WF1B_RING = 17            # extra fc1 ring slots in conv-freed SBUF


def _build_program(B_pc, G):
    """Emit the per-core Bass program. B_pc = samples per core, G = chunk."""
    bf = mybir.dt.bfloat16
    f32 = mybir.dt.float32
    nchunks = B_pc // G
    ngrp = G // 4          # 4-sample matmul groups per chunk
    NT = F1 // 128         # 8 fc1 row tiles
    NL = 2 * L3            # fc1 l-steps

    nc = bacc.Bacc("TRN2", target_bir_lowering=False, debug=False)

    xa_d = nc.dram_tensor("xa", [nchunks, 128, G * L1], bf, kind="ExternalInput")
    # x0 (64 rows) + ones row, all samples: [65, B_pc]
    x0_d = nc.dram_tensor("x0", [IL + 1, B_pc], bf, kind="ExternalInput")
    # conv1 weights: wa0b = [x0-tap weights; b1 row] (K=65), wxi = xi-tap
    # weights for the two cout tiles stacked on partition halves (K=64 each)
    wa0b_d = nc.dram_tensor("wa0b", [IL + 1, CH], bf, kind="ExternalInput")
    wxi_d = nc.dram_tensor("wxi", [128, 128], bf, kind="ExternalInput")
    w2_d = nc.dram_tensor("w2", [128, 12 * 128], bf, kind="ExternalInput")
    w3_d = nc.dram_tensor("w3", [128, 12 * 128], bf, kind="ExternalInput")
    wf1_d = nc.dram_tensor("wf1", [NL, 128, F1], bf, kind="ExternalInput")
    wf2_d = nc.dram_tensor("wf2", [128, NT * OUT], bf, kind="ExternalInput")
    # bias columns: 0:2 b1, 2:4 b2, 4:6 b3, 6:6+NT bfc1, then bfc2
    # broadcast as OUT columns
    bias_d = nc.dram_tensor("bias", [128, 6 + NT + OUT], f32, kind="ExternalInput")
    out_d = nc.dram_tensor("out", [B_pc, OUT], f32, kind="ExternalOutput")

    relu = mybir.ActivationFunctionType.Relu
    add_op = mybir.AluOpType.add
    max_op = mybir.AluOpType.max

    drain_ctr = [0]

    def drain(out_ap, in_ap, bias_ap):
        """relu(in + bias) -> out, alternating ACT / DVE (GPSIMD can't
        read PSUM)."""
        if drain_ctr[0] % 2 == 0:
            nc.scalar.activation(out_ap, in_ap, relu, bias=bias_ap)
        else:
            nc.vector.tensor_scalar(out_ap, in_ap, bias_ap, 0.0, add_op, max_op)
        drain_ctr[0] += 1

    with tile.TileContext(nc) as tc:
        with (
            tc.tile_pool(name="persist", bufs=1) as pp,
            tc.tile_pool(name="wf1", bufs=WF1_RING) as wfp,
        ):
            # startup DMAs: conv1 weights on scalar, first xa chunk split
            # sync/gpsimd
            warm_t = pp.tile([128, 128], bf, name="warm_t", tag="warm")
            nc.vector.memset(warm_t[:], 0.0)
            wa0b_t = pp.tile([IL + 1, CH], bf, name="wa0b_t", tag="wa0b")
            nc.scalar.dma_start(out=wa0b_t[:], in_=wa0b_d.ap())
            wxi_t = pp.tile([128, 128], bf, name="wxi_t", tag="wxi")
            nc.scalar.dma_start(out=wxi_t[:], in_=wxi_d.ap())
            x0_t = pp.tile([IL + 1, B_pc], bf, name="x0_t", tag="x0")
            nc.scalar.dma_start(out=x0_t[:], in_=x0_d.ap())
            # per-(cout,sample) x0 contribution incl. b1, for drain bias
            a_t = pp.tile([128, 2 * B_pc], f32, name="a_t", tag="a")
            bias_t = pp.tile([128, 6 + NT + OUT], f32, name="bias_t", tag="bias")
            nc.scalar.dma_start(out=bias_t[:], in_=bias_d.ap())
            w2_t = pp.tile([128, 12 * 128], bf, name="w2_t", tag="w2")
            w3_t = pp.tile([128, 12 * 128], bf, name="w3_t", tag="w3")
            wf2_t = pp.tile([128, NT * OUT], bf, name="wf2_t", tag="wf2")
            # conv3 output, resident, l-major: y3[ct][p, l*B_pc + s]
            y3_t = [pp.tile([128, B_pc * L3], bf, name=f"y3_{i}", tag=f"y3_{i}") for i in range(2)]
            # fc1 output (post-relu), nt-major columns
            out1_t = pp.tile([128, NT * B_pc], bf, name="out1_t", tag="out1")

            dma_engs = (nc.sync, nc.scalar, nc.gpsimd)

            # ring-head prefetch is staggered into conv chunks 1..3 so the
            # early xa chunk DMAs are not delayed
            wf_tiles = []

            def prefetch_wf1(n):
                for _ in range(n):
                    i = len(wf_tiles)
                    wt = wfp.tile([128, F1], bf, name="wf1_t", tag="wf1")
                    # hold prefetch transfers past the startup DMA crunch
                    with tc.tile_wait_until(0.022 + 0.003 * i):
                        dma_engs[i % 3].dma_start(out=wt[:], in_=wf1_d.ap()[i])
                    wf_tiles.append(wt)

            # ---- conv phase ----
            with (
                tc.tile_pool(name="xa", bufs=2) as xap,
                tc.tile_pool(name="y1", bufs=2) as y1p,
                tc.tile_pool(name="y2", bufs=1) as y2p,
                tc.tile_pool(name="cpsum", bufs=6, space="PSUM") as cps,
            ):
                for c in range(nchunks):
                    xat = xap.tile([128, G * L1], bf, name="xa_t", tag="xa")
                    if c == 0:
                        q = G * L1 // 8

                        def slice_dma(sl, eng):
                            eng.dma_start(
                                out=xat[:, sl * q:(sl + 1) * q],
                                in_=xa_d.ap()[c][:, sl * q:(sl + 1) * q])

                        for sl in range(8):
                            slice_dma(sl, nc.sync if sl % 2 == 0 else nc.gpsimd)
                        nc.gpsimd.dma_start(out=w2_t[:], in_=w2_d.ap())
                        nc.gpsimd.dma_start(out=w3_t[:], in_=w3_d.ap())
                    elif c == 1:
                        with tc.tile_wait_until(0.008):
                            nc.scalar.dma_start(out=xat[:, 0:G * L1 // 2],
                                                in_=xa_d.ap()[c][:, 0:G * L1 // 2])
                            nc.sync.dma_start(out=xat[:, G * L1 // 2:],
                                              in_=xa_d.ap()[c][:, G * L1 // 2:])
                    else:
                        nc.scalar.dma_start(out=xat[:], in_=xa_d.ap()[c])
                    if c == 0:
                        nc.gpsimd.dma_start(out=wf2_t[:], in_=wf2_d.ap())
                        wps = cps.tile([128, 4 * L1], f32, name="warm", tag="cps")
                        for _ in range(34):
                            nc.tensor.matmul(wps[:, 0:128], warm_t[:],
                                             warm_t[:], start=True, stop=True)
                    if 1 <= c <= 3:
                        n3 = WF1_RING // 3
                        prefetch_wf1(n3 if c < 3 else WF1_RING - 2 * n3)
                    y1t = [y1p.tile([128, G * L1], bf, name=f"y1t_{i}", tag=f"y1_{i}") for i in range(2)]
                    y2t = [y2p.tile([128, G * L2], bf, name=f"y2t_{i}", tag=f"y2_{i}") for i in range(2)]
                    y1v = [y1t[i][:].rearrange("p (s l) -> p s l", l=L1)
                           for i in range(2)]
                    y2v = [y2t[i][:].rearrange("p (s l) -> p s l", l=L2)
                           for i in range(2)]

                    def emit_a():
                        # x0 contribution for all 16 samples of this chunk:
                        # K=65 matmul (x0 taps + ones row folding in b1)
                        for ct in range(2):
                            aps = cps.tile([128, G], f32, name="cpsa",
                                           tag="cpsa", bufs=2)
                            nc.tensor.matmul(
                                aps[:],
                                wa0b_t[:, ct * 128:(ct + 1) * 128],
                                x0_t[:, c * G:(c + 1) * G],
                                start=True, stop=True,
                            )
                            dst = a_t[:, ct * B_pc + c * G:
                                      ct * B_pc + (c + 1) * G]
                            if ct == 0:
                                nc.vector.tensor_copy(out=dst, in_=aps[:])
                            else:
                                nc.scalar.activation(
                                    dst, aps[:],
                                    mybir.ActivationFunctionType.Identity)

                    def emit_c1(g):
                        # conv1: two concurrent K=64 row-tile matmuls (xi taps
                        # only); the x0 part enters via the per-sample drain
                        # bias
                        ps = [cps.tile([128, 4 * L1], f32, name=f"cps1_{i}",
                                       tag="cps") for i in range(2)]
                        for ct in range(2):
                            nc.tensor.matmul(
                                ps[ct][:],
                                wxi_t[64 * ct:64 * (ct + 1), :],
                                xat[64 * ct:64 * (ct + 1),
                                    g * 4 * L1:(g + 1) * 4 * L1],
                                start=True, stop=True,
                            )
                        for ct in range(2):
                            for s in range(4):
                                sg = c * G + 4 * g + s
                                drain(y1t[ct][:, (4 * g + s) * L1:
                                              (4 * g + s + 1) * L1],
                                      ps[ct][:, s * L1:(s + 1) * L1],
                                      a_t[:, ct * B_pc + sg:
                                          ct * B_pc + sg + 1])

                    def emit_c2(g, ct):
                        # conv2: 3x2 accumulating matmuls per (group, co_t)
                        ps = cps.tile([128, 4 * L2], f32, name="cps2", tag="cps")
                        for k in range(3):
                            for ci in range(2):
                                j = k * 4 + ci * 2 + ct
                                nc.tensor.matmul(
                                    ps[:],
                                    w2_t[:, j * 128:(j + 1) * 128],
                                    y1v[ci][:, 4 * g:4 * g + 4, k:k + L2],
                                    start=(k == 0 and ci == 0),
                                    stop=(k == 2 and ci == 1),
                                )
                        drain(y2t[ct][:, g * 4 * L2:(g + 1) * 4 * L2], ps[:],
                              bias_t[:, 2 + ct:3 + ct])

                    def emit_c3(g, ct):
                        # conv3: l-major resident y3; the drain writes
                        # [l, 4 contiguous samples] runs (8B) while reading
                        # the s-major PSUM through a strided (l, s) view
                        s0 = c * G + 4 * g
                        ps = cps.tile([128, 4 * L3], f32, name="cps3", tag="cps")
                        for k in range(3):
                            for ci in range(2):
                                j = k * 4 + ci * 2 + ct
                                nc.tensor.matmul(
                                    ps[:],
                                    w3_t[:, j * 128:(j + 1) * 128],
                                    y2v[ci][:, 4 * g:4 * g + 4, k:k + L3],
                                    start=(k == 0 and ci == 0),
                                    stop=(k == 2 and ci == 1),
                                )
                        y3v = y3_t[ct][:].rearrange("p (l s) -> p l s", s=B_pc)
                        psv = ps[:].rearrange("p (s l) -> p l s", l=L3)
                        h = 66
                        nc.scalar.activation(y3v[:, 0:h, s0:s0 + 4],
                                             psv[:, 0:h, :], relu,
                                             bias=bias_t[:, 4 + ct:5 + ct])
                        nc.vector.tensor_scalar(y3v[:, h:L3, s0:s0 + 4],
                                                psv[:, h:L3, :],
                                                bias_t[:, 4 + ct:5 + ct],
                                                0.0, add_op, max_op)

                    emit_a()
                    for g in range(ngrp):
                        emit_c1(g)
                    for g in range(ngrp):
                        for ct in range(2):
                            emit_c2(g, ct)
                    for g in range(ngrp):
                        for ct in range(2):
                            emit_c3(g, ct)

            # ---- fc1: stream weights through the ring, accumulate in PSUM;
            # fc2 chained off each psf drain. A second ring pool reuses the
            # SBUF freed by the conv pools, deepening the stream buffer ----
            with (
                tc.tile_pool(name="wf1b", bufs=WF1B_RING) as wfp2,
                tc.tile_pool(name="fpsum", bufs=1, space="PSUM") as fps,
            ):
                psf = [fps.tile([128, B_pc], f32, name=f"psf_{i}", tag=f"psf_{i}") for i in range(NT)]
                for i in range(NL):
                    ct, l = divmod(i, L3)
                    if i < WF1_RING:
                        wt = wf_tiles[i]
                    else:
                        pool = wfp if i % 2 == 0 else wfp2
                        wt = pool.tile([128, F1], bf, name="wf1_t", tag="wf1")
                        dma_engs[i % 3].dma_start(out=wt[:], in_=wf1_d.ap()[i])
                    rhs = y3_t[ct][:, l * B_pc:(l + 1) * B_pc]
                    for nt in range(NT):
                        nc.tensor.matmul(
                            psf[nt],
                            wt[:, nt * 128:(nt + 1) * 128],
                            rhs,
                            start=(i == 0),
                            stop=(i == NL - 1),
                        )
                for bh in range(B_pc // 128):
                    for nt in range(NT):
                        drain(out1_t[:, nt * B_pc + bh * 128:
                                     nt * B_pc + (bh + 1) * 128],
                              psf[nt][:, bh * 128:(bh + 1) * 128],
                              bias_t[:, 6 + nt:7 + nt])

            # fc2: per-bh chains consume out1 blocks right behind the psf
            # drains; psum [samples, OUT] keeps the output DMA contiguous
            with (
                tc.tile_pool(name="opsum", bufs=2, space="PSUM") as ops,
                tc.tile_pool(name="osb", bufs=2) as osb,
            ):
                for bh in range(B_pc // 128):
                    ps = ops.tile([128, OUT], f32, name="ops_t", tag="ops")
                    for nt in range(NT):
                        nc.tensor.matmul(
                            ps[:],
                            out1_t[:, nt * B_pc + bh * 128:
                                   nt * B_pc + (bh + 1) * 128],
                            wf2_t[:, nt * OUT:(nt + 1) * OUT],
                            start=(nt == 0),
                            stop=(nt == NT - 1),
                        )
                    ot = osb.tile([128, OUT], f32, name="osb_t", tag="osb")
                    nc.vector.tensor_tensor(
                        out=ot[:], in0=ps[:],
                        in1=bias_t[:, 6 + NT:6 + NT + OUT],
                        op=add_op,
                    )
                    (nc.sync if bh == 0 else nc.scalar).dma_start(
                        out=out_d.ap()[bh * 128:(bh + 1) * 128, :],
                        in_=ot[:])

    nc.compile()
    return nc


def _host_prep(x, w1, b1, w2, b2, w3, b3, wfc1, bfc1, wfc2, bfc2, B_pc, G):
    """Build per-core input maps (shared weight arrays built once)."""
    NT = F1 // 128
    nchunks = B_pc // G

    # conv1 input: xi taps duplicated on both partition halves (row-tile
    # packed K=64 matmuls); x0 handled separately via a K=65 matmul
    B = x.shape[0]
    xr = np.ascontiguousarray(x.reshape(B, CL, IL).transpose(0, 2, 1))  # [B, IL, CL]
    xa = np.empty((B, 128, L1), dtype=np.float32)
    xa[:, :IL, :] = xr[:, :, 1:]
    xa[:, IL:, :] = xr[:, :, 1:]
    xa = xa.astype(BF16)

    # x0 + ones row: [B, 65] -> per-core [65, B_pc]
    x0full = np.empty((B, IL + 1), dtype=np.float32)
    x0full[:, :IL] = xr[:, :, 0]
    x0full[:, IL] = 1.0
    x0full = x0full.astype(BF16)

    # conv1 weights: wa0b[r, c] = w1[c, r, 0] (r<64), row 64 = b1
    wa0b = np.empty((IL + 1, CH), dtype=np.float32)
    wa0b[:IL] = w1[:, :, 0].T
    wa0b[IL] = b1
    wa0b = np.ascontiguousarray(wa0b.astype(BF16))
    # wxi: xi-tap weights, cout tiles stacked on partition halves
    wxi = np.empty((128, 128), dtype=np.float32)
    wxi[:IL] = w1[0:128, :, 1].T
    wxi[IL:] = w1[128:256, :, 1].T
    wxi = np.ascontiguousarray(wxi.astype(BF16))

    def conv_tiles(w):
        # w [co, ci, k] -> [ci(128), j*128+co], j = k*4 + ci_t*2 + co_t
        t = w.reshape(2, 128, 2, 128, 3)  # [co_t, co, ci_t, ci, k]
        t = t.transpose(4, 2, 0, 3, 1)    # [k, ci_t, co_t, ci, co]
        t = t.reshape(12, 128, 128).transpose(1, 0, 2).reshape(128, 12 * 128)
        return np.ascontiguousarray(t.astype(BF16))

    w2sb = conv_tiles(w2)
    w3sb = conv_tiles(w3)

    # fc1 weights: wf1[ct*123+l][co, nt*128+n] = wfc1[nt*128+n, (ct*128+co)*123+l]
    t = wfc1.reshape(F1, 2, 128, L3)      # [n, co_t, co, l]
    t = t.transpose(1, 3, 2, 0)           # [co_t, l, co, n]
    wf1 = np.ascontiguousarray(t.reshape(2 * L3, 128, F1).astype(BF16))

    # fc2: wf2[n, nt*16+o] = wfc2[o, nt*128+n]
    t = wfc2.T.reshape(NT, 128, OUT).transpose(1, 0, 2).reshape(128, NT * OUT)
    wf2 = np.ascontiguousarray(t.astype(BF16))

    bias = np.zeros((128, 6 + NT + OUT), dtype=np.float32)
    bias[:, 0:2] = b1.reshape(2, 128).T
    bias[:, 2:4] = b2.reshape(2, 128).T
    bias[:, 4:6] = b3.reshape(2, 128).T
    bias[:, 6:6 + NT] = bfc1.reshape(NT, 128).T
    bias[:, 6 + NT:] = bfc2[None, :]

    in_maps = []
    ncores = B // B_pc
    for ci in range(ncores):
        shard = xa[ci * B_pc:(ci + 1) * B_pc]            # [B_pc, 128, L1]
        shard = shard.reshape(nchunks, G, 128, L1).transpose(0, 2, 1, 3)
        shard = np.ascontiguousarray(shard).reshape(nchunks, 128, G * L1)
        x0sh = np.ascontiguousarray(x0full[ci * B_pc:(ci + 1) * B_pc].T)
        in_maps.append({
            "xa": shard, "x0": x0sh, "wa0b": wa0b, "wxi": wxi,
            "w2": w2sb, "w3": w3sb,
            "wf1": wf1, "wf2": wf2, "bias": bias,
        })
    return in_maps


def kernel(x, w1, b1, w2, b2, w3, b3, wfc1, bfc1, wfc2, bfc2):
    B_pc = BATCH // NCORES
    G = 16
    key = ("prog", B_pc, G)
    if key not in _CACHE:
        _CACHE[key] = _build_program(B_pc, G)
    nc = _CACHE[key]
    in_maps = _host_prep(
        np.asarray(x, dtype=np.float32), np.asarray(w1, dtype=np.float32),
        np.asarray(b1, dtype=np.float32), np.asarray(w2, dtype=np.float32),
        np.asarray(b2, dtype=np.float32), np.asarray(w3, dtype=np.float32),
        np.asarray(b3, dtype=np.float32), np.asarray(wfc1, dtype=np.float32),
        np.asarray(bfc1, dtype=np.float32), np.asarray(wfc2, dtype=np.float32),
        np.asarray(bfc2, dtype=np.float32), B_pc, G,
    )
    trace = bool(os.environ.get("KERNEL_TRACE"))
    res = run_bass_kernel_spmd(nc, in_maps, core_ids=list(range(NCORES)),
                               trace=trace)
    _CACHE["last_results"] = res
    return np.concatenate([res.results[i]["out"] for i in range(NCORES)], axis=0)



# revision 31
# speedup vs baseline: 1.1056x; 1.1056x over previous
"""Trainium2 Bass kernel for nn_CNN2_P (dense CNN + MLP head).

Pure data-parallel over 8 NeuronCores: batch 2048 -> 256 per core, all
weights replicated. Host-side prep re-tiles weights into PE-friendly
layouts and casts to bf16; the device kernel runs conv1/2/3 as
accumulating matmuls (channels on partitions). conv3's drain writes the
resident l-major y3 through an [l, 4-contiguous-samples] view (8-byte
runs, cheap) while reading the sample-major PSUM strided, so fc1's rhs
slices stay fully contiguous (a strided matmul rhs runs ~4x slow on the
PE). The streamed fc1 weights flow through a 20-slot SBUF ring whose
head is prefetched during the conv phase; fc2 chains off the fc1 PSUM
drains per batch-half. Warm-up matmuls on a memset tile ramp the PE
DVFS p-state through the startup DMA latency, and DMA triggers are
placed so weight/chunk transfers self-pace behind the drain queues.
"""

import os

import numpy as np
import ml_dtypes

import concourse.mybir as mybir
import concourse.bacc as bacc
import concourse.tile as tile
from concourse.bass_utils import run_bass_kernel_spmd

# Problem constants (hardcoded per contract).
CL, IL = 128, 64          # context length, instruction length
CH = 256                  # channels in all three convs
L1, L2, L3 = 127, 125, 123
F1, OUT = 1024, 16
BATCH = 2048
NCORES = 8

BF16 = ml_dtypes.bfloat16

_CACHE = {}

WF1_RING = 18             # persistent SBUF ring depth for fc1 weights
WF1B_RING = 17            # extra fc1 ring slots in conv-freed SBUF


def _build_program(B_pc, G):
    """Emit the per-core Bass program. B_pc = samples per core, G = chunk."""
    bf = mybir.dt.bfloat16
    f32 = mybir.dt.float32
    nchunks = B_pc // G
    ngrp = G // 4          # 4-sample matmul groups per chunk
    NT = F1 // 128         # 8 fc1 row tiles
    NL = 2 * L3            # fc1 l-steps

    nc = bacc.Bacc("TRN2", target_bir_lowering=False, debug=False)

    xa_d = nc.dram_tensor("xa", [nchunks, 128, G * L1], bf, kind="ExternalInput")
    # x0 (64 rows) + ones row, all samples: [65, B_pc]
    x0_d = nc.dram_tensor("x0", [IL + 1, B_pc], bf, kind="ExternalInput")
    # conv1 weights: wa0b = [x0-tap weights; b1 row] (K=65), wxi = xi-tap
    # weights for the two cout tiles stacked on partition halves (K=64 each)
    wa0b_d = nc.dram_tensor("wa0b", [IL + 1, CH], bf, kind="ExternalInput")
    wxi_d = nc.dram_tensor("wxi", [128, 128], bf, kind="ExternalInput")
    w2_d = nc.dram_tensor("w2", [128, 12 * 128], bf, kind="ExternalInput")
    w3_d = nc.dram_tensor("w3", [128, 12 * 128], bf, kind="ExternalInput")
    wf1_d = nc.dram_tensor("wf1", [NL, 128, F1], bf, kind="ExternalInput")
    wf2_d = nc.dram_tensor("wf2", [128, NT * OUT], bf, kind="ExternalInput")
    # bias columns: 0:2 b1, 2:4 b2, 4:6 b3, 6:6+NT bfc1, then bfc2
    # broadcast as OUT columns
    bias_d = nc.dram_tensor("bias", [128, 6 + NT + OUT], f32, kind="ExternalInput")
    out_d = nc.dram_tensor("out", [B_pc, OUT], f32, kind="ExternalOutput")

    relu = mybir.ActivationFunctionType.Relu
    add_op = mybir.AluOpType.add
    max_op = mybir.AluOpType.max

    drain_ctr = [0]

    def drain(out_ap, in_ap, bias_ap):
        """relu(in + bias) -> out, alternating ACT / DVE (GPSIMD can't
        read PSUM)."""
        if drain_ctr[0] % 2 == 0:
            nc.scalar.activation(out_ap, in_ap, relu, bias=bias_ap)
        else:
            nc.vector.tensor_scalar(out_ap, in_ap, bias_ap, 0.0, add_op, max_op)
        drain_ctr[0] += 1

    with tile.TileContext(nc) as tc:
        with (
            tc.tile_pool(name="persist", bufs=1) as pp,
            tc.tile_pool(name="wf1", bufs=WF1_RING) as wfp,
        ):
            # startup DMAs: conv1 weights on scalar, first xa chunk split
            # sync/gpsimd
            warm_t = pp.tile([128, 128], bf, name="warm_t", tag="warm")
            nc.vector.memset(warm_t[:], 0.0)
            wa0b_t = pp.tile([IL + 1, CH], bf, name="wa0b_t", tag="wa0b")
            nc.scalar.dma_start(out=wa0b_t[:], in_=wa0b_d.ap())
            wxi_t = pp.tile([128, 128], bf, name="wxi_t", tag="wxi")
            nc.scalar.dma_start(out=wxi_t[:], in_=wxi_d.ap())
            x0_t = pp.tile([IL + 1, B_pc], bf, name="x0_t", tag="x0")
            nc.scalar.dma_start(out=x0_t[:], in_=x0_d.ap())
            # per-(cout,sample) x0 contribution incl. b1, for drain bias
            a_t = pp.tile([128, 2 * B_pc], f32, name="a_t", tag="a")
            bias_t = pp.tile([128, 6 + NT + OUT], f32, name="bias_t", tag="bias")
            nc.scalar.dma_start(out=bias_t[:], in_=bias_d.ap())
            w2_t = pp.tile([128, 12 * 128], bf, name="w2_t", tag="w2")
            w3_t = pp.tile([128, 12 * 128], bf, name="w3_t", tag="w3")
            wf2_t = pp.tile([128, NT * OUT], bf, name="wf2_t", tag="wf2")
            # conv3 output, resident, l-major: y3[ct][p, l*B_pc + s]
            y3_t = [pp.tile([128, B_pc * L3], bf, name=f"y3_{i}", tag=f"y3_{i}") for i in range(2)]
            # fc1 output (post-relu), nt-major columns
            out1_t = pp.tile([128, NT * B_pc], bf, name="out1_t", tag="out1")

            dma_engs = (nc.sync, nc.scalar, nc.gpsimd)

            # ring-head prefetch is staggered into conv chunks 1..3 so the
            # early xa chunk DMAs are not delayed
            wf_tiles = []

            def prefetch_wf1(n):
                for _ in range(n):
                    i = len(wf_tiles)
                    wt = wfp.tile([128, F1], bf, name="wf1_t", tag="wf1")
                    # hold prefetch transfers past the startup DMA crunch
                    with tc.tile_wait_until(0.022 + 0.003 * i):
                        dma_engs[i % 3].dma_start(out=wt[:], in_=wf1_d.ap()[i])
                    wf_tiles.append(wt)

            # ---- conv phase ----
            with (
                tc.tile_pool(name="xa", bufs=2) as xap,
                tc.tile_pool(name="y1", bufs=2) as y1p,
                tc.tile_pool(name="y2", bufs=1) as y2p,
                tc.tile_pool(name="cpsum", bufs=6, space="PSUM") as cps,
            ):
                for c in range(nchunks):
                    xat = xap.tile([128, G * L1], bf, name="xa_t", tag="xa")
                    if c == 0:
                        q = G * L1 // 8

                        def slice_dma(sl, eng):
                            eng.dma_start(
                                out=xat[:, sl * q:(sl + 1) * q],
                                in_=xa_d.ap()[c][:, sl * q:(sl + 1) * q])

                        for sl in range(8):
                            slice_dma(sl, nc.sync if sl % 2 == 0 else nc.gpsimd)
                        nc.gpsimd.dma_start(out=w2_t[:], in_=w2_d.ap())
                        nc.gpsimd.dma_start(out=w3_t[:], in_=w3_d.ap())
                    elif c == 1:
                        with tc.tile_wait_until(0.008):
                            nc.scalar.dma_start(out=xat[:, 0:G * L1 // 2],
                                                in_=xa_d.ap()[c][:, 0:G * L1 // 2])
                            nc.sync.dma_start(out=xat[:, G * L1 // 2:],
                                              in_=xa_d.ap()[c][:, G * L1 // 2:])
                    else:
                        nc.scalar.dma_start(out=xat[:], in_=xa_d.ap()[c])
                    if c == 0:
                        nc.gpsimd.dma_start(out=wf2_t[:], in_=wf2_d.ap())
                        wps = cps.tile([128, 4 * L1], f32, name="warm", tag="cps")
                        for _ in range(34):
                            nc.tensor.matmul(wps[:, 0:128], warm_t[:],
                                             warm_t[:], start=True, stop=True)
                    if 1 <= c <= 3:
                        n3 = WF1_RING // 3
                        prefetch_wf1(n3 if c < 3 else WF1_RING - 2 * n3)
                    y1t = [y1p.tile([128, G * L1], bf, name=f"y1t_{i}", tag=f"y1_{i}") for i in range(2)]
                    y2t = [y2p.tile([128, G * L2], bf, name=f"y2t_{i}", tag=f"y2_{i}") for i in range(2)]
                    y1v = [y1t[i][:].rearrange("p (s l) -> p s l", l=L1)
                           for i in range(2)]
                    y2v = [y2t[i][:].rearrange("p (s l) -> p s l", l=L2)
                           for i in range(2)]

                    def emit_a():
                        # x0 contribution for all 16 samples of this chunk:
                        # K=65 matmul (x0 taps + ones row folding in b1)
                        for ct in range(2):
                            aps = cps.tile([128, G], f32, name="cpsa",
                                           tag="cpsa", bufs=2)
                            nc.tensor.matmul(
                                aps[:],
                                wa0b_t[:, ct * 128:(ct + 1) * 128],
                                x0_t[:, c * G:(c + 1) * G],
                                start=True, stop=True,
                            )
                            dst = a_t[:, ct * B_pc + c * G:
                                      ct * B_pc + (c + 1) * G]
                            nc.vector.tensor_copy(out=dst, in_=aps[:])

                    def emit_c1(g):
                        # conv1: two concurrent K=64 row-tile matmuls (xi taps
                        # only); the x0 part enters via the per-sample drain
                        # bias
                        ps = [cps.tile([128, 4 * L1], f32, name=f"cps1_{i}",
                                       tag="cps") for i in range(2)]
                        for ct in range(2):
                            nc.tensor.matmul(
                                ps[ct][:],
                                wxi_t[64 * ct:64 * (ct + 1), :],
                                xat[64 * ct:64 * (ct + 1),
                                    g * 4 * L1:(g + 1) * 4 * L1],
                                start=True, stop=True,
                            )
                        for ct in range(2):
                            for s in range(4):
                                sg = c * G + 4 * g + s
                                nc.vector.tensor_scalar(
                                    y1t[ct][:, (4 * g + s) * L1:
                                            (4 * g + s + 1) * L1],
                                    ps[ct][:, s * L1:(s + 1) * L1],
                                    a_t[:, ct * B_pc + sg:
                                        ct * B_pc + sg + 1],
                                    0.0, add_op, max_op)

                    def emit_c2(g, ct):
                        # conv2: 3x2 accumulating matmuls per (group, co_t)
                        ps = cps.tile([128, 4 * L2], f32, name="cps2", tag="cps")
                        for k in range(3):
                            for ci in range(2):
                                j = k * 4 + ci * 2 + ct
                                nc.tensor.matmul(
                                    ps[:],
                                    w2_t[:, j * 128:(j + 1) * 128],
                                    y1v[ci][:, 4 * g:4 * g + 4, k:k + L2],
                                    start=(k == 0 and ci == 0),
                                    stop=(k == 2 and ci == 1),
                                )
                        drain(y2t[ct][:, g * 4 * L2:(g + 1) * 4 * L2], ps[:],
                              bias_t[:, 2 + ct:3 + ct])

                    def emit_c3(g, ct):
                        # conv3: l-major resident y3; the drain writes
                        # [l, 4 contiguous samples] runs (8B) while reading
                        # the s-major PSUM through a strided (l, s) view
                        s0 = c * G + 4 * g
                        ps = cps.tile([128, 4 * L3], f32, name="cps3", tag="cps")
                        for k in range(3):
                            for ci in range(2):
                                j = k * 4 + ci * 2 + ct
                                nc.tensor.matmul(
                                    ps[:],
                                    w3_t[:, j * 128:(j + 1) * 128],
                                    y2v[ci][:, 4 * g:4 * g + 4, k:k + L3],
                                    start=(k == 0 and ci == 0),
                                    stop=(k == 2 and ci == 1),
                                )
                        y3v = y3_t[ct][:].rearrange("p (l s) -> p l s", s=B_pc)
                        psv = ps[:].rearrange("p (s l) -> p l s", l=L3)
                        h = 66
                        nc.scalar.activation(y3v[:, 0:h, s0:s0 + 4],
                                             psv[:, 0:h, :], relu,
                                             bias=bias_t[:, 4 + ct:5 + ct])
                        nc.vector.tensor_scalar(y3v[:, h:L3, s0:s0 + 4],
                                                psv[:, h:L3, :],
                                                bias_t[:, 4 + ct:5 + ct],
                                                0.0, add_op, max_op)

                    emit_a()
                    for g in range(ngrp):
                        emit_c1(g)
                    for g in range(ngrp):
                        for ct in range(2):
                            emit_c2(g, ct)
                    for g in range(ngrp):
                        for ct in range(2):
                            emit_c3(g, ct)

            # ---- fc1: stream weights through the ring, accumulate in PSUM;
            # fc2 chained off each psf drain. A second ring pool reuses the
            # SBUF freed by the conv pools, deepening the stream buffer ----
            with (
                tc.tile_pool(name="wf1b", bufs=WF1B_RING) as wfp2,
                tc.tile_pool(name="fpsum", bufs=1, space="PSUM") as fps,
            ):
                psf = [fps.tile([128, B_pc], f32, name=f"psf_{i}", tag=f"psf_{i}") for i in range(NT)]
                for i in range(NL):
                    ct, l = divmod(i, L3)
                    if i < WF1_RING:
                        wt = wf_tiles[i]
                    else:
                        pool = wfp if i % 2 == 0 else wfp2
                        wt = pool.tile([128, F1], bf, name="wf1_t", tag="wf1")
                        dma_engs[i % 3].dma_start(out=wt[:], in_=wf1_d.ap()[i])
                    rhs = y3_t[ct][:, l * B_pc:(l + 1) * B_pc]
                    for nt in range(NT):
                        nc.tensor.matmul(
                            psf[nt],
                            wt[:, nt * 128:(nt + 1) * 128],
                            rhs,
                            start=(i == 0),
                            stop=(i == NL - 1),
                        )
                for bh in range(B_pc // 128):
                    for nt in range(NT):
                        drain(out1_t[:, nt * B_pc + bh * 128:
                                     nt * B_pc + (bh + 1) * 128],
                              psf[nt][:, bh * 128:(bh + 1) * 128],
                              bias_t[:, 6 + nt:7 + nt])

            # fc2: per-bh chains consume out1 blocks right behind the psf
            # drains; psum [samples, OUT] keeps the output DMA contiguous
            with (
                tc.tile_pool(name="opsum", bufs=2, space="PSUM") as ops,
                tc.tile_pool(name="osb", bufs=2) as osb,
            ):
                for bh in range(B_pc // 128):
                    ps = ops.tile([128, OUT], f32, name="ops_t", tag="ops")
                    for nt in range(NT):
                        nc.tensor.matmul(
                            ps[:],
                            out1_t[:, nt * B_pc + bh * 128:
                                   nt * B_pc + (bh + 1) * 128],
                            wf2_t[:, nt * OUT:(nt + 1) * OUT],
                            start=(nt == 0),
                            stop=(nt == NT - 1),
                        )
                    ot = osb.tile([128, OUT], f32, name="osb_t", tag="osb")
                    nc.vector.tensor_tensor(
                        out=ot[:], in0=ps[:],
                        in1=bias_t[:, 6 + NT:6 + NT + OUT],
                        op=add_op,
                    )
                    (nc.sync if bh == 0 else nc.scalar).dma_start(
                        out=out_d.ap()[bh * 128:(bh + 1) * 128, :],
                        in_=ot[:])

    nc.compile()
    return nc


def _host_prep(x, w1, b1, w2, b2, w3, b3, wfc1, bfc1, wfc2, bfc2, B_pc, G):
    """Build per-core input maps (shared weight arrays built once)."""
    NT = F1 // 128
    nchunks = B_pc // G

    # conv1 input: xi taps duplicated on both partition halves (row-tile
    # packed K=64 matmuls); x0 handled separately via a K=65 matmul
    B = x.shape[0]
    xr = np.ascontiguousarray(x.reshape(B, CL, IL).transpose(0, 2, 1))  # [B, IL, CL]
    xa = np.empty((B, 128, L1), dtype=np.float32)
    xa[:, :IL, :] = xr[:, :, 1:]
    xa[:, IL:, :] = xr[:, :, 1:]
    xa = xa.astype(BF16)

    # x0 + ones row: [B, 65] -> per-core [65, B_pc]
    x0full = np.empty((B, IL + 1), dtype=np.float32)
    x0full[:, :IL] = xr[:, :, 0]
    x0full[:, IL] = 1.0
    x0full = x0full.astype(BF16)

    # conv1 weights: wa0b[r, c] = w1[c, r, 0] (r<64), row 64 = b1
    wa0b = np.empty((IL + 1, CH), dtype=np.float32)
    wa0b[:IL] = w1[:, :, 0].T
    wa0b[IL] = b1
    wa0b = np.ascontiguousarray(wa0b.astype(BF16))
    # wxi: xi-tap weights, cout tiles stacked on partition halves
    wxi = np.empty((128, 128), dtype=np.float32)
    wxi[:IL] = w1[0:128, :, 1].T
    wxi[IL:] = w1[128:256, :, 1].T
    wxi = np.ascontiguousarray(wxi.astype(BF16))

    def conv_tiles(w):
        # w [co, ci, k] -> [ci(128), j*128+co], j = k*4 + ci_t*2 + co_t
        t = w.reshape(2, 128, 2, 128, 3)  # [co_t, co, ci_t, ci, k]
        t = t.transpose(4, 2, 0, 3, 1)    # [k, ci_t, co_t, ci, co]
        t = t.reshape(12, 128, 128).transpose(1, 0, 2).reshape(128, 12 * 128)
        return np.ascontiguousarray(t.astype(BF16))

    w2sb = conv_tiles(w2)
    w3sb = conv_tiles(w3)

    # fc1 weights: wf1[ct*123+l][co, nt*128+n] = wfc1[nt*128+n, (ct*128+co)*123+l]
    t = wfc1.reshape(F1, 2, 128, L3)      # [n, co_t, co, l]
    t = t.transpose(1, 3, 2, 0)           # [co_t, l, co, n]
    wf1 = np.ascontiguousarray(t.reshape(2 * L3, 128, F1).astype(BF16))

    # fc2: wf2[n, nt*16+o] = wfc2[o, nt*128+n]
    t = wfc2.T.reshape(NT, 128, OUT).transpose(1, 0, 2).reshape(128, NT * OUT)
    wf2 = np.ascontiguousarray(t.astype(BF16))

    bias = np.zeros((128, 6 + NT + OUT), dtype=np.float32)
    bias[:, 0:2] = b1.reshape(2, 128).T
    bias[:, 2:4] = b2.reshape(2, 128).T
    bias[:, 4:6] = b3.reshape(2, 128).T
    bias[:, 6:6 + NT] = bfc1.reshape(NT, 128).T
    bias[:, 6 + NT:] = bfc2[None, :]

    in_maps = []
    ncores = B // B_pc
    for ci in range(ncores):
        shard = xa[ci * B_pc:(ci + 1) * B_pc]            # [B_pc, 128, L1]
        shard = shard.reshape(nchunks, G, 128, L1).transpose(0, 2, 1, 3)
        shard = np.ascontiguousarray(shard).reshape(nchunks, 128, G * L1)
        x0sh = np.ascontiguousarray(x0full[ci * B_pc:(ci + 1) * B_pc].T)
        in_maps.append({
            "xa": shard, "x0": x0sh, "wa0b": wa0b, "wxi": wxi,
            "w2": w2sb, "w3": w3sb,
            "wf1": wf1, "wf2": wf2, "bias": bias,
        })
    return in_maps


def kernel(x, w1, b1, w2, b2, w3, b3, wfc1, bfc1, wfc2, bfc2):
    B_pc = BATCH // NCORES
    G = 16
    key = ("prog", B_pc, G)
    if key not in _CACHE:
        _CACHE[key] = _build_program(B_pc, G)
    nc = _CACHE[key]
    in_maps = _host_prep(
        np.asarray(x, dtype=np.float32), np.asarray(w1, dtype=np.float32),
        np.asarray(b1, dtype=np.float32), np.asarray(w2, dtype=np.float32),
        np.asarray(b2, dtype=np.float32), np.asarray(w3, dtype=np.float32),
        np.asarray(b3, dtype=np.float32), np.asarray(wfc1, dtype=np.float32),
        np.asarray(bfc1, dtype=np.float32), np.asarray(wfc2, dtype=np.float32),
        np.asarray(bfc2, dtype=np.float32), B_pc, G,
    )
    trace = bool(os.environ.get("KERNEL_TRACE"))
    res = run_bass_kernel_spmd(nc, in_maps, core_ids=list(range(NCORES)),
                               trace=trace)
    _CACHE["last_results"] = res
    return np.concatenate([res.results[i]["out"] for i in range(NCORES)], axis=0)



# revision 32
# speedup vs baseline: 1.2475x; 1.1284x over previous
"""Trainium2 Bass kernel for nn_CNN2_P (dense CNN + MLP head).

Pure data-parallel over 8 NeuronCores: batch 2048 -> 256 per core, all
weights replicated. Host-side prep re-tiles weights into PE-friendly
layouts and casts to bf16; the device kernel runs conv1/2/3 as
accumulating matmuls (channels on partitions). conv3's drain writes the
resident l-major y3 through an [l, 4-contiguous-samples] view (8-byte
runs, cheap) while reading the sample-major PSUM strided, so fc1's rhs
slices stay fully contiguous (a strided matmul rhs runs ~4x slow on the
PE). The streamed fc1 weights flow through a 20-slot SBUF ring whose
head is prefetched during the conv phase; fc2 chains off the fc1 PSUM
drains per batch-half. Warm-up matmuls on a memset tile ramp the PE
DVFS p-state through the startup DMA latency, and DMA triggers are
placed so weight/chunk transfers self-pace behind the drain queues.
"""

import os

import numpy as np
import ml_dtypes

import concourse.mybir as mybir
import concourse.bacc as bacc
import concourse.tile as tile
from concourse.bass_utils import run_bass_kernel_spmd

# Problem constants (hardcoded per contract).
CL, IL = 128, 64          # context length, instruction length
CH = 256                  # channels in all three convs
L1, L2, L3 = 127, 125, 123
F1, OUT = 1024, 16
BATCH = 2048
NCORES = 8

BF16 = ml_dtypes.bfloat16

_CACHE = {}

WF1_RING = 20             # persistent SBUF ring depth for fc1 weights
WF1B_RING = 16            # extra fc1 ring slots in conv-freed SBUF


def _build_program(B_pc, G):
    """Emit the per-core Bass program. B_pc = samples per core, G = chunk."""
    bf = mybir.dt.bfloat16
    f32 = mybir.dt.float32
    nchunks = B_pc // G
    ngrp = G // 4          # 4-sample matmul groups per chunk
    NT = F1 // 128         # 8 fc1 row tiles
    NL = 2 * L3            # fc1 l-steps

    nc = bacc.Bacc("TRN2", target_bir_lowering=False, debug=False)

    xa_d = nc.dram_tensor("xa", [nchunks, 128, G * L1], bf, kind="ExternalInput")
    wa_d = nc.dram_tensor("wa", [128, CH], bf, kind="ExternalInput")
    w2_d = nc.dram_tensor("w2", [128, 12 * 128], bf, kind="ExternalInput")
    w3_d = nc.dram_tensor("w3", [128, 12 * 128], bf, kind="ExternalInput")
    wf1_d = nc.dram_tensor("wf1", [NL, 128, F1], bf, kind="ExternalInput")
    wf2_d = nc.dram_tensor("wf2", [128, NT * OUT], bf, kind="ExternalInput")
    # bias columns: 0:2 b1, 2:4 b2, 4:6 b3, 6:6+NT bfc1, then bfc2
    # broadcast as OUT columns
    bias_d = nc.dram_tensor("bias", [128, 6 + NT + OUT], f32, kind="ExternalInput")
    out_d = nc.dram_tensor("out", [B_pc, OUT], f32, kind="ExternalOutput")

    relu = mybir.ActivationFunctionType.Relu
    add_op = mybir.AluOpType.add
    max_op = mybir.AluOpType.max

    drain_ctr = [0]

    def drain(out_ap, in_ap, bias_ap):
        """relu(in + bias) -> out, alternating ACT / DVE (GPSIMD can't
        read PSUM)."""
        if drain_ctr[0] % 2 == 0:
            nc.scalar.activation(out_ap, in_ap, relu, bias=bias_ap)
        else:
            nc.vector.tensor_scalar(out_ap, in_ap, bias_ap, 0.0, add_op, max_op)
        drain_ctr[0] += 1

    with tile.TileContext(nc) as tc:
        with (
            tc.tile_pool(name="persist", bufs=1) as pp,
            tc.tile_pool(name="wf1", bufs=WF1_RING) as wfp,
        ):
            # startup DMAs: wa on scalar, first xa chunk split sync/gpsimd
            warm_t = pp.tile([128, 128], bf, name="warm_t", tag="warm")
            nc.vector.memset(warm_t[:], 0.0)
            wa_t = pp.tile([128, CH], bf, name="wa_t", tag="wa")
            nc.scalar.dma_start(out=wa_t[:, 0:128], in_=wa_d.ap()[:, 0:128])
            nc.scalar.dma_start(out=wa_t[:, 128:CH], in_=wa_d.ap()[:, 128:CH])
            bias_t = pp.tile([128, 6 + NT + OUT], f32, name="bias_t", tag="bias")
            nc.scalar.dma_start(out=bias_t[:], in_=bias_d.ap())
            w2_t = pp.tile([128, 12 * 128], bf, name="w2_t", tag="w2")
            w3_t = pp.tile([128, 12 * 128], bf, name="w3_t", tag="w3")
            wf2_t = pp.tile([128, NT * OUT], bf, name="wf2_t", tag="wf2")
            # conv3 output, resident, l-major: y3[ct][p, l*B_pc + s]
            y3_t = [pp.tile([128, B_pc * L3], bf, name=f"y3_{i}", tag=f"y3_{i}") for i in range(2)]
            # fc1 output (post-relu), nt-major columns
            out1_t = pp.tile([128, NT * B_pc], bf, name="out1_t", tag="out1")

            dma_engs = (nc.sync, nc.scalar, nc.gpsimd)

            # ring-head prefetch is staggered into conv chunks 1..3 so the
            # early xa chunk DMAs are not delayed
            wf_tiles = []

            def prefetch_wf1(n):
                for _ in range(n):
                    i = len(wf_tiles)
                    wt = wfp.tile([128, F1], bf, name="wf1_t", tag="wf1")
                    # hold prefetch transfers past the startup DMA crunch
                    with tc.tile_wait_until(0.022 + 0.003 * i):
                        dma_engs[i % 3].dma_start(out=wt[:], in_=wf1_d.ap()[i])
                    wf_tiles.append(wt)

            # ---- conv phase ----
            with (
                tc.tile_pool(name="xa", bufs=2) as xap,
                tc.tile_pool(name="y1", bufs=2) as y1p,
                tc.tile_pool(name="y2", bufs=1) as y2p,
                tc.tile_pool(name="cpsum", bufs=8, space="PSUM") as cps,
            ):
                for c in range(nchunks):
                    xat = xap.tile([128, G * L1], bf, name="xa_t", tag="xa")
                    if c == 0:
                        q = G * L1 // 8

                        def slice_dma(sl, eng):
                            eng.dma_start(
                                out=xat[:, sl * q:(sl + 1) * q],
                                in_=xa_d.ap()[c][:, sl * q:(sl + 1) * q])

                        for sl in range(8):
                            slice_dma(sl, nc.sync if sl % 2 == 0 else nc.gpsimd)
                        nc.gpsimd.dma_start(out=w2_t[:], in_=w2_d.ap())
                        nc.gpsimd.dma_start(out=w3_t[:], in_=w3_d.ap())
                    elif c == 1:
                        with tc.tile_wait_until(0.008):
                            nc.scalar.dma_start(out=xat[:, 0:G * L1 // 2],
                                                in_=xa_d.ap()[c][:, 0:G * L1 // 2])
                            nc.sync.dma_start(out=xat[:, G * L1 // 2:],
                                              in_=xa_d.ap()[c][:, G * L1 // 2:])
                    else:
                        nc.scalar.dma_start(out=xat[:], in_=xa_d.ap()[c])
                    if c == 0:
                        nc.gpsimd.dma_start(out=wf2_t[:], in_=wf2_d.ap())
                        wps = cps.tile([128, 4 * L1], f32, name="warm", tag="cps")
                        for _ in range(34):
                            nc.tensor.matmul(wps[:, 0:128], warm_t[:],
                                             warm_t[:], start=True, stop=True)
                    if 1 <= c <= 3:
                        n3 = WF1_RING // 3
                        prefetch_wf1(n3 if c < 3 else WF1_RING - 2 * n3)
                    y1t = [y1p.tile([128, G * L1], bf, name=f"y1t_{i}", tag=f"y1_{i}") for i in range(2)]
                    y2t = [y2p.tile([128, G * L2], bf, name=f"y2t_{i}", tag=f"y2_{i}") for i in range(2)]
                    y1v = [y1t[i][:].rearrange("p (s l) -> p s l", l=L1)
                           for i in range(2)]
                    y2v = [y2t[i][:].rearrange("p (s l) -> p s l", l=L2)
                           for i in range(2)]

                    def emit_c1(g, ct):
                        # conv1: augmented K=128 matmul, N = 4*L1
                        ps = cps.tile([128, 4 * L1], f32, name="cps1", tag="cps")
                        nc.tensor.matmul(
                            ps[:],
                            wa_t[:, ct * 128:(ct + 1) * 128],
                            xat[:, g * 4 * L1:(g + 1) * 4 * L1],
                            start=True, stop=True,
                        )
                        drain(y1t[ct][:, g * 4 * L1:(g + 1) * 4 * L1], ps[:],
                              bias_t[:, ct:ct + 1])

                    def emit_c2(g, ct):
                        # conv2: 3x2 accumulating matmuls per (group, co_t)
                        ps = cps.tile([128, 4 * L2], f32, name="cps2", tag="cps")
                        for k in range(3):
                            for ci in range(2):
                                j = k * 4 + ci * 2 + ct
                                nc.tensor.matmul(
                                    ps[:],
                                    w2_t[:, j * 128:(j + 1) * 128],
                                    y1v[ci][:, 4 * g:4 * g + 4, k:k + L2],
                                    start=(k == 0 and ci == 0),
                                    stop=(k == 2 and ci == 1),
                                )
                        drain(y2t[ct][:, g * 4 * L2:(g + 1) * 4 * L2], ps[:],
                              bias_t[:, 2 + ct:3 + ct])

                    def emit_c3(g, ct):
                        # conv3: l-major resident y3; the drain writes
                        # [l, 4 contiguous samples] runs (8B) while reading
                        # the s-major PSUM through a strided (l, s) view
                        s0 = c * G + 4 * g
                        ps = cps.tile([128, 4 * L3], f32, name="cps3", tag="cps")
                        for k in range(3):
                            for ci in range(2):
                                j = k * 4 + ci * 2 + ct
                                nc.tensor.matmul(
                                    ps[:],
                                    w3_t[:, j * 128:(j + 1) * 128],
                                    y2v[ci][:, 4 * g:4 * g + 4, k:k + L3],
                                    start=(k == 0 and ci == 0),
                                    stop=(k == 2 and ci == 1),
                                )
                        y3v = y3_t[ct][:].rearrange("p (l s) -> p l s", s=B_pc)
                        psv = ps[:].rearrange("p (s l) -> p l s", l=L3)
                        h = 66
                        nc.scalar.activation(y3v[:, 0:h, s0:s0 + 4],
                                             psv[:, 0:h, :], relu,
                                             bias=bias_t[:, 4 + ct:5 + ct])
                        nc.vector.tensor_scalar(y3v[:, h:L3, s0:s0 + 4],
                                                psv[:, h:L3, :],
                                                bias_t[:, 4 + ct:5 + ct],
                                                0.0, add_op, max_op)

                    for g in range(ngrp):
                        for ct in range(2):
                            emit_c1(g, ct)
                    for g in range(ngrp):
                        for ct in range(2):
                            emit_c2(g, ct)
                    for g in range(ngrp):
                        for ct in range(2):
                            emit_c3(g, ct)

            # ---- fc1: stream weights through the ring, accumulate in PSUM;
            # fc2 chained off each psf drain. A second ring pool reuses the
            # SBUF freed by the conv pools, deepening the stream buffer ----
            with (
                tc.tile_pool(name="wf1b", bufs=WF1B_RING) as wfp2,
                tc.tile_pool(name="fpsum", bufs=1, space="PSUM") as fps,
            ):
                psf = [fps.tile([128, B_pc], f32, name=f"psf_{i}", tag=f"psf_{i}") for i in range(NT)]
                for i in range(NL):
                    ct, l = divmod(i, L3)
                    if i < WF1_RING:
                        wt = wf_tiles[i]
                    else:
                        pool = wfp if i % 2 == 0 else wfp2
                        wt = pool.tile([128, F1], bf, name="wf1_t", tag="wf1")
                        dma_engs[i % 3].dma_start(out=wt[:], in_=wf1_d.ap()[i])
                    rhs = y3_t[ct][:, l * B_pc:(l + 1) * B_pc]
                    for nt in range(NT):
                        nc.tensor.matmul(
                            psf[nt],
                            wt[:, nt * 128:(nt + 1) * 128],
                            rhs,
                            start=(i == 0),
                            stop=(i == NL - 1),
                        )
                for bh in range(B_pc // 128):
                    for nt in range(NT):
                        drain(out1_t[:, nt * B_pc + bh * 128:
                                     nt * B_pc + (bh + 1) * 128],
                              psf[nt][:, bh * 128:(bh + 1) * 128],
                              bias_t[:, 6 + nt:7 + nt])

            # fc2: per-bh chains consume out1 blocks right behind the psf
            # drains; psum [samples, OUT] keeps the output DMA contiguous
            with (
                tc.tile_pool(name="opsum", bufs=2, space="PSUM") as ops,
                tc.tile_pool(name="osb", bufs=2) as osb,
            ):
                for bh in range(B_pc // 128):
                    ps = ops.tile([128, OUT], f32, name="ops_t", tag="ops")
                    for nt in range(NT):
                        nc.tensor.matmul(
                            ps[:],
                            out1_t[:, nt * B_pc + bh * 128:
                                   nt * B_pc + (bh + 1) * 128],
                            wf2_t[:, nt * OUT:(nt + 1) * OUT],
                            start=(nt == 0),
                            stop=(nt == NT - 1),
                        )
                    ot = osb.tile([128, OUT], f32, name="osb_t", tag="osb")
                    nc.vector.tensor_tensor(
                        out=ot[:], in0=ps[:],
                        in1=bias_t[:, 6 + NT:6 + NT + OUT],
                        op=add_op,
                    )
                    (nc.sync if bh == 0 else nc.scalar).dma_start(
                        out=out_d.ap()[bh * 128:(bh + 1) * 128, :],
                        in_=ot[:])

    nc.compile()
    return nc


def _host_prep(x, w1, b1, w2, b2, w3, b3, wfc1, bfc1, wfc2, bfc2, B_pc, G):
    """Build per-core input maps (shared weight arrays built once)."""
    NT = F1 // 128
    nchunks = B_pc // G

    # Augmented conv1 input: rows 0..63 = x0 broadcast, 64..127 = xr[:, :, 1:]
    B = x.shape[0]
    xr = np.ascontiguousarray(x.reshape(B, CL, IL).transpose(0, 2, 1))  # [B, IL, CL]
    xa = np.empty((B, 128, L1), dtype=np.float32)
    xa[:, :IL, :] = xr[:, :, 0:1]
    xa[:, IL:, :] = xr[:, :, 1:]
    xa = xa.astype(BF16)

    # conv1 weights: watilde[r, c] = w1[c, r, 0] (r<64) else w1[c, r-64, 1]
    wa = np.concatenate([w1[:, :, 0].T, w1[:, :, 1].T], axis=0).astype(BF16)
    wa = np.ascontiguousarray(wa)  # [128, 256]

    def conv_tiles(w):
        # w [co, ci, k] -> [ci(128), j*128+co], j = k*4 + ci_t*2 + co_t
        t = w.reshape(2, 128, 2, 128, 3)  # [co_t, co, ci_t, ci, k]
        t = t.transpose(4, 2, 0, 3, 1)    # [k, ci_t, co_t, ci, co]
        t = t.reshape(12, 128, 128).transpose(1, 0, 2).reshape(128, 12 * 128)
        return np.ascontiguousarray(t.astype(BF16))

    w2sb = conv_tiles(w2)
    w3sb = conv_tiles(w3)

    # fc1 weights: wf1[ct*123+l][co, nt*128+n] = wfc1[nt*128+n, (ct*128+co)*123+l]
    t = wfc1.reshape(F1, 2, 128, L3)      # [n, co_t, co, l]
    t = t.transpose(1, 3, 2, 0)           # [co_t, l, co, n]
    wf1 = np.ascontiguousarray(t.reshape(2 * L3, 128, F1).astype(BF16))

    # fc2: wf2[n, nt*16+o] = wfc2[o, nt*128+n]
    t = wfc2.T.reshape(NT, 128, OUT).transpose(1, 0, 2).reshape(128, NT * OUT)
    wf2 = np.ascontiguousarray(t.astype(BF16))

    bias = np.zeros((128, 6 + NT + OUT), dtype=np.float32)
    bias[:, 0:2] = b1.reshape(2, 128).T
    bias[:, 2:4] = b2.reshape(2, 128).T
    bias[:, 4:6] = b3.reshape(2, 128).T
    bias[:, 6:6 + NT] = bfc1.reshape(NT, 128).T
    bias[:, 6 + NT:] = bfc2[None, :]

    in_maps = []
    ncores = B // B_pc
    for ci in range(ncores):
        shard = xa[ci * B_pc:(ci + 1) * B_pc]            # [B_pc, 128, L1]
        shard = shard.reshape(nchunks, G, 128, L1).transpose(0, 2, 1, 3)
        shard = np.ascontiguousarray(shard).reshape(nchunks, 128, G * L1)
        in_maps.append({
            "xa": shard, "wa": wa, "w2": w2sb, "w3": w3sb,
            "wf1": wf1, "wf2": wf2, "bias": bias,
        })
    return in_maps


def kernel(x, w1, b1, w2, b2, w3, b3, wfc1, bfc1, wfc2, bfc2):
    B_pc = BATCH // NCORES
    G = 16
    key = ("prog", B_pc, G)
    if key not in _CACHE:
        _CACHE[key] = _build_program(B_pc, G)
    nc = _CACHE[key]
    in_maps = _host_prep(
        np.asarray(x, dtype=np.float32), np.asarray(w1, dtype=np.float32),
        np.asarray(b1, dtype=np.float32), np.asarray(w2, dtype=np.float32),
        np.asarray(b2, dtype=np.float32), np.asarray(w3, dtype=np.float32),
        np.asarray(b3, dtype=np.float32), np.asarray(wfc1, dtype=np.float32),
        np.asarray(bfc1, dtype=np.float32), np.asarray(wfc2, dtype=np.float32),
        np.asarray(bfc2, dtype=np.float32), B_pc, G,
    )
    trace = bool(os.environ.get("KERNEL_TRACE"))
    res = run_bass_kernel_spmd(nc, in_maps, core_ids=list(range(NCORES)),
                               trace=trace)
    _CACHE["last_results"] = res
    return np.concatenate([res.results[i]["out"] for i in range(NCORES)], axis=0)



# revision 33
# speedup vs baseline: 1.2530x; 1.0043x over previous
"""Trainium2 Bass kernel for nn_CNN2_P (dense CNN + MLP head).

Pure data-parallel over 8 NeuronCores: batch 2048 -> 256 per core, all
weights replicated. Host-side prep re-tiles weights into PE-friendly
layouts and casts to bf16; the device kernel runs conv1/2/3 as
accumulating matmuls (channels on partitions). conv3's drain writes the
resident l-major y3 through an [l, 4-contiguous-samples] view (8-byte
runs, cheap) while reading the sample-major PSUM strided, so fc1's rhs
slices stay fully contiguous (a strided matmul rhs runs ~4x slow on the
PE). The streamed fc1 weights flow through a 20-slot SBUF ring whose
head is prefetched during the conv phase; fc2 chains off the fc1 PSUM
drains per batch-half. Warm-up matmuls on a memset tile ramp the PE
DVFS p-state through the startup DMA latency, and DMA triggers are
placed so weight/chunk transfers self-pace behind the drain queues.
"""

import os

import numpy as np
import ml_dtypes

import concourse.mybir as mybir
import concourse.bacc as bacc
import concourse.tile as tile
from concourse.bass_utils import run_bass_kernel_spmd

# Problem constants (hardcoded per contract).
CL, IL = 128, 64          # context length, instruction length
CH = 256                  # channels in all three convs
L1, L2, L3 = 127, 125, 123
F1, OUT = 1024, 16
BATCH = 2048
NCORES = 8

BF16 = ml_dtypes.bfloat16

_CACHE = {}

WF1_RING = 20             # persistent SBUF ring depth for fc1 weights
WF1B_RING = 16            # extra fc1 ring slots in conv-freed SBUF


def _build_program(B_pc, G):
    """Emit the per-core Bass program. B_pc = samples per core, G = chunk."""
    bf = mybir.dt.bfloat16
    f32 = mybir.dt.float32
    nchunks = B_pc // G
    ngrp = G // 4          # 4-sample matmul groups per chunk
    NT = F1 // 128         # 8 fc1 row tiles
    NL = 2 * L3            # fc1 l-steps

    nc = bacc.Bacc("TRN2", target_bir_lowering=False, debug=False)

    xa_d = nc.dram_tensor("xa", [nchunks, 128, G * L1], bf, kind="ExternalInput")
    wa_d = nc.dram_tensor("wa", [128, CH], bf, kind="ExternalInput")
    w2_d = nc.dram_tensor("w2", [128, 12 * 128], bf, kind="ExternalInput")
    w3_d = nc.dram_tensor("w3", [128, 12 * 128], bf, kind="ExternalInput")
    wf1_d = nc.dram_tensor("wf1", [NL, 128, F1], bf, kind="ExternalInput")
    wf2_d = nc.dram_tensor("wf2", [128, NT * OUT], bf, kind="ExternalInput")
    # bias columns: 0:2 b1, 2:4 b2, 4:6 b3, 6:6+NT bfc1, then bfc2
    # broadcast as OUT columns
    bias_d = nc.dram_tensor("bias", [128, 6 + NT + OUT], f32, kind="ExternalInput")
    out_d = nc.dram_tensor("out", [B_pc, OUT], f32, kind="ExternalOutput")

    relu = mybir.ActivationFunctionType.Relu
    add_op = mybir.AluOpType.add
    max_op = mybir.AluOpType.max

    drain_ctr = [0]

    def drain(out_ap, in_ap, bias_ap):
        """relu(in + bias) -> out, alternating ACT / DVE (GPSIMD can't
        read PSUM)."""
        if drain_ctr[0] % 2 == 0:
            nc.scalar.activation(out_ap, in_ap, relu, bias=bias_ap)
        else:
            nc.vector.tensor_scalar(out_ap, in_ap, bias_ap, 0.0, add_op, max_op)
        drain_ctr[0] += 1

    with tile.TileContext(nc) as tc:
        with (
            tc.tile_pool(name="persist", bufs=1) as pp,
            tc.tile_pool(name="wf1", bufs=WF1_RING) as wfp,
        ):
            # startup DMAs: wa on scalar, first xa chunk split sync/gpsimd
            warm_t = pp.tile([128, 128], bf, name="warm_t", tag="warm")
            nc.vector.memset(warm_t[:], 0.0)
            wa_t = pp.tile([128, CH], bf, name="wa_t", tag="wa")
            nc.scalar.dma_start(out=wa_t[:, 0:128], in_=wa_d.ap()[:, 0:128])
            nc.scalar.dma_start(out=wa_t[:, 128:CH], in_=wa_d.ap()[:, 128:CH])
            bias_t = pp.tile([128, 6 + NT + OUT], f32, name="bias_t", tag="bias")
            nc.scalar.dma_start(out=bias_t[:], in_=bias_d.ap())
            w2_t = pp.tile([128, 12 * 128], bf, name="w2_t", tag="w2")
            w3_t = pp.tile([128, 12 * 128], bf, name="w3_t", tag="w3")
            wf2_t = pp.tile([128, NT * OUT], bf, name="wf2_t", tag="wf2")
            # conv3 output, resident, l-major: y3[ct][p, l*B_pc + s]
            y3_t = [pp.tile([128, B_pc * L3], bf, name=f"y3_{i}", tag=f"y3_{i}") for i in range(2)]
            # fc1 output (post-relu), nt-major columns
            out1_t = pp.tile([128, NT * B_pc], bf, name="out1_t", tag="out1")

            dma_engs = (nc.sync, nc.scalar, nc.gpsimd)

            # ring-head prefetch is staggered into conv chunks 1..3 so the
            # early xa chunk DMAs are not delayed
            wf_tiles = []

            def prefetch_wf1(n):
                for _ in range(n):
                    i = len(wf_tiles)
                    wt = wfp.tile([128, F1], bf, name="wf1_t", tag="wf1")
                    # hold prefetch transfers past the startup DMA crunch
                    with tc.tile_wait_until(0.022 + 0.003 * i):
                        dma_engs[i % 3].dma_start(out=wt[:], in_=wf1_d.ap()[i])
                    wf_tiles.append(wt)

            # ---- conv phase ----
            with (
                tc.tile_pool(name="xa", bufs=2) as xap,
                tc.tile_pool(name="y1", bufs=2) as y1p,
                tc.tile_pool(name="y2", bufs=1) as y2p,
                tc.tile_pool(name="cpsum", bufs=8, space="PSUM") as cps,
            ):
                for c in range(nchunks):
                    xat = xap.tile([128, G * L1], bf, name="xa_t", tag="xa")
                    if c == 0:
                        q = G * L1 // 8

                        def slice_dma(sl, eng):
                            eng.dma_start(
                                out=xat[:, sl * q:(sl + 1) * q],
                                in_=xa_d.ap()[c][:, sl * q:(sl + 1) * q])

                        for sl in range(8):
                            slice_dma(sl, nc.sync if sl % 2 == 0 else nc.gpsimd)
                        # w2 must land before conv2(c0) (~10us): split across
                        # scalar (idle after wa/bias) + sync
                        hw = 6 * 128
                        nc.scalar.dma_start(out=w2_t[:, 0:hw],
                                            in_=w2_d.ap()[:, 0:hw])
                        nc.sync.dma_start(out=w2_t[:, hw:],
                                          in_=w2_d.ap()[:, hw:])
                        nc.scalar.dma_start(out=w3_t[:, 0:hw],
                                            in_=w3_d.ap()[:, 0:hw])
                        nc.gpsimd.dma_start(out=w3_t[:, hw:],
                                            in_=w3_d.ap()[:, hw:])
                    elif c == 1:
                        with tc.tile_wait_until(0.008):
                            nc.scalar.dma_start(out=xat[:, 0:G * L1 // 2],
                                                in_=xa_d.ap()[c][:, 0:G * L1 // 2])
                            nc.sync.dma_start(out=xat[:, G * L1 // 2:],
                                              in_=xa_d.ap()[c][:, G * L1 // 2:])
                    else:
                        nc.scalar.dma_start(out=xat[:], in_=xa_d.ap()[c])
                    if c == 0:
                        nc.gpsimd.dma_start(out=wf2_t[:], in_=wf2_d.ap())
                        wps = cps.tile([128, 4 * L1], f32, name="warm", tag="cps")
                        for _ in range(34):
                            nc.tensor.matmul(wps[:, 0:128], warm_t[:],
                                             warm_t[:], start=True, stop=True)
                    if 1 <= c <= 3:
                        n3 = WF1_RING // 3
                        prefetch_wf1(n3 if c < 3 else WF1_RING - 2 * n3)
                    y1t = [y1p.tile([128, G * L1], bf, name=f"y1t_{i}", tag=f"y1_{i}") for i in range(2)]
                    y2t = [y2p.tile([128, G * L2], bf, name=f"y2t_{i}", tag=f"y2_{i}") for i in range(2)]
                    y1v = [y1t[i][:].rearrange("p (s l) -> p s l", l=L1)
                           for i in range(2)]
                    y2v = [y2t[i][:].rearrange("p (s l) -> p s l", l=L2)
                           for i in range(2)]

                    def emit_c1(g, ct):
                        # conv1: augmented K=128 matmul, N = 4*L1
                        ps = cps.tile([128, 4 * L1], f32, name="cps1", tag="cps")
                        nc.tensor.matmul(
                            ps[:],
                            wa_t[:, ct * 128:(ct + 1) * 128],
                            xat[:, g * 4 * L1:(g + 1) * 4 * L1],
                            start=True, stop=True,
                        )
                        drain(y1t[ct][:, g * 4 * L1:(g + 1) * 4 * L1], ps[:],
                              bias_t[:, ct:ct + 1])

                    def emit_c2(g, ct):
                        # conv2: 3x2 accumulating matmuls per (group, co_t)
                        ps = cps.tile([128, 4 * L2], f32, name="cps2", tag="cps")
                        for k in range(3):
                            for ci in range(2):
                                j = k * 4 + ci * 2 + ct
                                nc.tensor.matmul(
                                    ps[:],
                                    w2_t[:, j * 128:(j + 1) * 128],
                                    y1v[ci][:, 4 * g:4 * g + 4, k:k + L2],
                                    start=(k == 0 and ci == 0),
                                    stop=(k == 2 and ci == 1),
                                )
                        drain(y2t[ct][:, g * 4 * L2:(g + 1) * 4 * L2], ps[:],
                              bias_t[:, 2 + ct:3 + ct])

                    def emit_c3(g, ct):
                        # conv3: l-major resident y3; the drain writes
                        # [l, 4 contiguous samples] runs (8B) while reading
                        # the s-major PSUM through a strided (l, s) view
                        s0 = c * G + 4 * g
                        ps = cps.tile([128, 4 * L3], f32, name="cps3", tag="cps")
                        for k in range(3):
                            for ci in range(2):
                                j = k * 4 + ci * 2 + ct
                                nc.tensor.matmul(
                                    ps[:],
                                    w3_t[:, j * 128:(j + 1) * 128],
                                    y2v[ci][:, 4 * g:4 * g + 4, k:k + L3],
                                    start=(k == 0 and ci == 0),
                                    stop=(k == 2 and ci == 1),
                                )
                        y3v = y3_t[ct][:].rearrange("p (l s) -> p l s", s=B_pc)
                        psv = ps[:].rearrange("p (s l) -> p l s", l=L3)
                        h = 66
                        nc.scalar.activation(y3v[:, 0:h, s0:s0 + 4],
                                             psv[:, 0:h, :], relu,
                                             bias=bias_t[:, 4 + ct:5 + ct])
                        nc.vector.tensor_scalar(y3v[:, h:L3, s0:s0 + 4],
                                                psv[:, h:L3, :],
                                                bias_t[:, 4 + ct:5 + ct],
                                                0.0, add_op, max_op)

                    for g in range(ngrp):
                        for ct in range(2):
                            emit_c1(g, ct)
                    for g in range(ngrp):
                        for ct in range(2):
                            emit_c2(g, ct)
                    for g in range(ngrp):
                        for ct in range(2):
                            emit_c3(g, ct)

            # ---- fc1: stream weights through the ring, accumulate in PSUM;
            # fc2 chained off each psf drain. A second ring pool reuses the
            # SBUF freed by the conv pools, deepening the stream buffer ----
            with (
                tc.tile_pool(name="wf1b", bufs=WF1B_RING) as wfp2,
                tc.tile_pool(name="fpsum", bufs=1, space="PSUM") as fps,
            ):
                psf = [fps.tile([128, B_pc], f32, name=f"psf_{i}", tag=f"psf_{i}") for i in range(NT)]
                for i in range(NL):
                    ct, l = divmod(i, L3)
                    if i < WF1_RING:
                        wt = wf_tiles[i]
                    else:
                        pool = wfp if i % 2 == 0 else wfp2
                        wt = pool.tile([128, F1], bf, name="wf1_t", tag="wf1")
                        dma_engs[i % 3].dma_start(out=wt[:], in_=wf1_d.ap()[i])
                    rhs = y3_t[ct][:, l * B_pc:(l + 1) * B_pc]
                    for nt in range(NT):
                        nc.tensor.matmul(
                            psf[nt],
                            wt[:, nt * 128:(nt + 1) * 128],
                            rhs,
                            start=(i == 0),
                            stop=(i == NL - 1),
                        )
                for bh in range(B_pc // 128):
                    for nt in range(NT):
                        drain(out1_t[:, nt * B_pc + bh * 128:
                                     nt * B_pc + (bh + 1) * 128],
                              psf[nt][:, bh * 128:(bh + 1) * 128],
                              bias_t[:, 6 + nt:7 + nt])

            # fc2: per-bh chains consume out1 blocks right behind the psf
            # drains; psum [samples, OUT] keeps the output DMA contiguous
            with (
                tc.tile_pool(name="opsum", bufs=2, space="PSUM") as ops,
                tc.tile_pool(name="osb", bufs=2) as osb,
            ):
                for bh in range(B_pc // 128):
                    ps = ops.tile([128, OUT], f32, name="ops_t", tag="ops")
                    for nt in range(NT):
                        nc.tensor.matmul(
                            ps[:],
                            out1_t[:, nt * B_pc + bh * 128:
                                   nt * B_pc + (bh + 1) * 128],
                            wf2_t[:, nt * OUT:(nt + 1) * OUT],
                            start=(nt == 0),
                            stop=(nt == NT - 1),
                        )
                    ot = osb.tile([128, OUT], f32, name="osb_t", tag="osb")
                    nc.vector.tensor_tensor(
                        out=ot[:], in0=ps[:],
                        in1=bias_t[:, 6 + NT:6 + NT + OUT],
                        op=add_op,
                    )
                    (nc.sync if bh == 0 else nc.scalar).dma_start(
                        out=out_d.ap()[bh * 128:(bh + 1) * 128, :],
                        in_=ot[:])

    nc.compile()
    return nc


def _host_prep(x, w1, b1, w2, b2, w3, b3, wfc1, bfc1, wfc2, bfc2, B_pc, G):
    """Build per-core input maps (shared weight arrays built once)."""
    NT = F1 // 128
    nchunks = B_pc // G

    # Augmented conv1 input: rows 0..63 = x0 broadcast, 64..127 = xr[:, :, 1:]
    B = x.shape[0]
    xr = np.ascontiguousarray(x.reshape(B, CL, IL).transpose(0, 2, 1))  # [B, IL, CL]
    xa = np.empty((B, 128, L1), dtype=np.float32)
    xa[:, :IL, :] = xr[:, :, 0:1]
    xa[:, IL:, :] = xr[:, :, 1:]
    xa = xa.astype(BF16)

    # conv1 weights: watilde[r, c] = w1[c, r, 0] (r<64) else w1[c, r-64, 1]
    wa = np.concatenate([w1[:, :, 0].T, w1[:, :, 1].T], axis=0).astype(BF16)
    wa = np.ascontiguousarray(wa)  # [128, 256]

    def conv_tiles(w):
        # w [co, ci, k] -> [ci(128), j*128+co], j = k*4 + ci_t*2 + co_t
        t = w.reshape(2, 128, 2, 128, 3)  # [co_t, co, ci_t, ci, k]
        t = t.transpose(4, 2, 0, 3, 1)    # [k, ci_t, co_t, ci, co]
        t = t.reshape(12, 128, 128).transpose(1, 0, 2).reshape(128, 12 * 128)
        return np.ascontiguousarray(t.astype(BF16))

    w2sb = conv_tiles(w2)
    w3sb = conv_tiles(w3)

    # fc1 weights: wf1[ct*123+l][co, nt*128+n] = wfc1[nt*128+n, (ct*128+co)*123+l]
    t = wfc1.reshape(F1, 2, 128, L3)      # [n, co_t, co, l]
    t = t.transpose(1, 3, 2, 0)           # [co_t, l, co, n]
    wf1 = np.ascontiguousarray(t.reshape(2 * L3, 128, F1).astype(BF16))

    # fc2: wf2[n, nt*16+o] = wfc2[o, nt*128+n]
    t = wfc2.T.reshape(NT, 128, OUT).transpose(1, 0, 2).reshape(128, NT * OUT)
    wf2 = np.ascontiguousarray(t.astype(BF16))

    bias = np.zeros((128, 6 + NT + OUT), dtype=np.float32)
    bias[:, 0:2] = b1.reshape(2, 128).T
    bias[:, 2:4] = b2.reshape(2, 128).T
    bias[:, 4:6] = b3.reshape(2, 128).T
    bias[:, 6:6 + NT] = bfc1.reshape(NT, 128).T
    bias[:, 6 + NT:] = bfc2[None, :]

    in_maps = []
    ncores = B // B_pc
    for ci in range(ncores):
        shard = xa[ci * B_pc:(ci + 1) * B_pc]            # [B_pc, 128, L1]
        shard = shard.reshape(nchunks, G, 128, L1).transpose(0, 2, 1, 3)
        shard = np.ascontiguousarray(shard).reshape(nchunks, 128, G * L1)
        in_maps.append({
            "xa": shard, "wa": wa, "w2": w2sb, "w3": w3sb,
            "wf1": wf1, "wf2": wf2, "bias": bias,
        })
    return in_maps


def kernel(x, w1, b1, w2, b2, w3, b3, wfc1, bfc1, wfc2, bfc2):
    B_pc = BATCH // NCORES
    G = 16
    key = ("prog", B_pc, G)
    if key not in _CACHE:
        _CACHE[key] = _build_program(B_pc, G)
    nc = _CACHE[key]
    in_maps = _host_prep(
        np.asarray(x, dtype=np.float32), np.asarray(w1, dtype=np.float32),
        np.asarray(b1, dtype=np.float32), np.asarray(w2, dtype=np.float32),
        np.asarray(b2, dtype=np.float32), np.asarray(w3, dtype=np.float32),
        np.asarray(b3, dtype=np.float32), np.asarray(wfc1, dtype=np.float32),
        np.asarray(bfc1, dtype=np.float32), np.asarray(wfc2, dtype=np.float32),
        np.asarray(bfc2, dtype=np.float32), B_pc, G,
    )
    trace = bool(os.environ.get("KERNEL_TRACE"))
    res = run_bass_kernel_spmd(nc, in_maps, core_ids=list(range(NCORES)),
                               trace=trace)
    _CACHE["last_results"] = res
    return np.concatenate([res.results[i]["out"] for i in range(NCORES)], axis=0)



# revision 36
# speedup vs baseline: 1.2537x; 1.0006x over previous
"""Trainium2 Bass kernel for nn_CNN2_P (dense CNN + MLP head).

Pure data-parallel over 8 NeuronCores: batch 2048 -> 256 per core, all
weights replicated. Host-side prep re-tiles weights into PE-friendly
layouts and casts to bf16; the device kernel runs conv1/2/3 as
accumulating matmuls (channels on partitions). conv3's drain writes the
resident l-major y3 through an [l, 4-contiguous-samples] view (8-byte
runs, cheap) while reading the sample-major PSUM strided, so fc1's rhs
slices stay fully contiguous (a strided matmul rhs runs ~4x slow on the
PE). The streamed fc1 weights flow through a 20-slot SBUF ring whose
head is prefetched during the conv phase; fc2 chains off the fc1 PSUM
drains per batch-half. Warm-up matmuls on a memset tile ramp the PE
DVFS p-state through the startup DMA latency, and DMA triggers are
placed so weight/chunk transfers self-pace behind the drain queues.
"""

import os

import numpy as np
import ml_dtypes

import concourse.mybir as mybir
import concourse.bacc as bacc
import concourse.tile as tile
from concourse.bass_utils import run_bass_kernel_spmd

# Problem constants (hardcoded per contract).
CL, IL = 128, 64          # context length, instruction length
CH = 256                  # channels in all three convs
L1, L2, L3 = 127, 125, 123
F1, OUT = 1024, 16
BATCH = 2048
NCORES = 8

BF16 = ml_dtypes.bfloat16

_CACHE = {}

WF1_RING = 20             # persistent SBUF ring depth for fc1 weights
WF1B_RING = 16            # extra fc1 ring slots in conv-freed SBUF


def _build_program(B_pc, G):
    """Emit the per-core Bass program. B_pc = samples per core, G = chunk."""
    bf = mybir.dt.bfloat16
    f32 = mybir.dt.float32
    nchunks = B_pc // G
    ngrp = G // 4          # 4-sample matmul groups per chunk
    NT = F1 // 128         # 8 fc1 row tiles
    NL = 2 * L3            # fc1 l-steps

    nc = bacc.Bacc("TRN2", target_bir_lowering=False, debug=False)

    xa_d = nc.dram_tensor("xa", [nchunks, 128, G * L1], bf, kind="ExternalInput")
    wa_d = nc.dram_tensor("wa", [128, CH], bf, kind="ExternalInput")
    w2_d = nc.dram_tensor("w2", [128, 12 * 128], bf, kind="ExternalInput")
    w3_d = nc.dram_tensor("w3", [128, 12 * 128], bf, kind="ExternalInput")
    wf1_d = nc.dram_tensor("wf1", [NL, 128, F1], bf, kind="ExternalInput")
    wf2_d = nc.dram_tensor("wf2", [128, NT * OUT], bf, kind="ExternalInput")
    # bias columns: 0:2 b1, 2:4 b2, 4:6 b3, 6:6+NT bfc1, then bfc2
    # broadcast as OUT columns
    bias_d = nc.dram_tensor("bias", [128, 6 + NT + OUT], f32, kind="ExternalInput")
    out_d = nc.dram_tensor("out", [B_pc, OUT], f32, kind="ExternalOutput")

    relu = mybir.ActivationFunctionType.Relu
    add_op = mybir.AluOpType.add
    max_op = mybir.AluOpType.max

    drain_ctr = [0]

    def drain(out_ap, in_ap, bias_ap):
        """relu(in + bias) -> out, alternating ACT / DVE (GPSIMD can't
        read PSUM)."""
        if drain_ctr[0] % 2 == 0:
            nc.scalar.activation(out_ap, in_ap, relu, bias=bias_ap)
        else:
            nc.vector.tensor_scalar(out_ap, in_ap, bias_ap, 0.0, add_op, max_op)
        drain_ctr[0] += 1

    with tile.TileContext(nc) as tc:
        with (
            tc.tile_pool(name="persist", bufs=1) as pp,
            tc.tile_pool(name="wf1", bufs=WF1_RING) as wfp,
        ):
            # startup DMAs: wa on scalar, first xa chunk split sync/gpsimd
            warm_t = pp.tile([128, 128], bf, name="warm_t", tag="warm")
            nc.vector.memset(warm_t[:], 0.0)
            wa_t = pp.tile([128, CH], bf, name="wa_t", tag="wa")
            nc.scalar.dma_start(out=wa_t[:, 0:128], in_=wa_d.ap()[:, 0:128])
            nc.scalar.dma_start(out=wa_t[:, 128:CH], in_=wa_d.ap()[:, 128:CH])
            bias_t = pp.tile([128, 6 + NT + OUT], f32, name="bias_t", tag="bias")
            nc.scalar.dma_start(out=bias_t[:], in_=bias_d.ap())
            w2_t = pp.tile([128, 12 * 128], bf, name="w2_t", tag="w2")
            w3_t = pp.tile([128, 12 * 128], bf, name="w3_t", tag="w3")
            wf2_t = pp.tile([128, NT * OUT], bf, name="wf2_t", tag="wf2")
            # conv3 output, resident, l-major: y3[ct][p, l*B_pc + s]
            y3_t = [pp.tile([128, B_pc * L3], bf, name=f"y3_{i}", tag=f"y3_{i}") for i in range(2)]
            # fc1 output (post-relu), nt-major columns
            out1_t = pp.tile([128, NT * B_pc], bf, name="out1_t", tag="out1")
            # fc2 output staging (both batch halves)
            osb_t = pp.tile([128, 2 * OUT], f32, name="osb_t", tag="osb")

            dma_engs = (nc.sync, nc.scalar, nc.gpsimd)

            # ring-head prefetch is staggered into conv chunks 1..3 so the
            # early xa chunk DMAs are not delayed
            wf_tiles = []

            def prefetch_wf1(n):
                for _ in range(n):
                    i = len(wf_tiles)
                    wt = wfp.tile([128, F1], bf, name="wf1_t", tag="wf1")
                    # hold prefetch transfers past the startup DMA crunch
                    with tc.tile_wait_until(0.022 + 0.003 * i):
                        dma_engs[i % 3].dma_start(out=wt[:], in_=wf1_d.ap()[i])
                    wf_tiles.append(wt)

            # ---- conv phase ----
            with (
                tc.tile_pool(name="xa", bufs=2) as xap,
                tc.tile_pool(name="y1", bufs=2) as y1p,
                tc.tile_pool(name="y2", bufs=1) as y2p,
                tc.tile_pool(name="cpsum", bufs=8, space="PSUM") as cps,
            ):
                for c in range(nchunks):
                    xat = xap.tile([128, G * L1], bf, name="xa_t", tag="xa")
                    if c == 0:
                        q = G * L1 // 8

                        def slice_dma(sl, eng):
                            eng.dma_start(
                                out=xat[:, sl * q:(sl + 1) * q],
                                in_=xa_d.ap()[c][:, sl * q:(sl + 1) * q])

                        for sl in range(8):
                            slice_dma(sl, nc.sync if sl % 2 == 0 else nc.gpsimd)
                        # w2 on scalar (idle after wa/bias, lands ~10us for
                        # conv2(c0)); w3 split on sync/gpsimd behind the xa
                        # slices (needed ~17us)
                        hw = 6 * 128
                        nc.scalar.dma_start(out=w2_t[:], in_=w2_d.ap())
                        nc.sync.dma_start(out=w3_t[:, 0:hw],
                                          in_=w3_d.ap()[:, 0:hw])
                        nc.gpsimd.dma_start(out=w3_t[:, hw:],
                                            in_=w3_d.ap()[:, hw:])
                    elif c == 1:
                        with tc.tile_wait_until(0.008):
                            nc.scalar.dma_start(out=xat[:, 0:G * L1 // 2],
                                                in_=xa_d.ap()[c][:, 0:G * L1 // 2])
                            nc.sync.dma_start(out=xat[:, G * L1 // 2:],
                                              in_=xa_d.ap()[c][:, G * L1 // 2:])
                    else:
                        nc.scalar.dma_start(out=xat[:], in_=xa_d.ap()[c])
                    if c == 0:
                        nc.gpsimd.dma_start(out=wf2_t[:], in_=wf2_d.ap())
                        wps = cps.tile([128, 4 * L1], f32, name="warm", tag="cps")
                        for _ in range(34):
                            nc.tensor.matmul(wps[:, 0:128], warm_t[:],
                                             warm_t[:], start=True, stop=True)
                    if 1 <= c <= 3:
                        n3 = WF1_RING // 3
                        prefetch_wf1(n3 if c < 3 else WF1_RING - 2 * n3)
                    y1t = [y1p.tile([128, G * L1], bf, name=f"y1t_{i}", tag=f"y1_{i}") for i in range(2)]
                    y2t = [y2p.tile([128, G * L2], bf, name=f"y2t_{i}", tag=f"y2_{i}") for i in range(2)]
                    y1v = [y1t[i][:].rearrange("p (s l) -> p s l", l=L1)
                           for i in range(2)]
                    y2v = [y2t[i][:].rearrange("p (s l) -> p s l", l=L2)
                           for i in range(2)]

                    def emit_c1(g, ct):
                        # conv1: augmented K=128 matmul, N = 4*L1
                        ps = cps.tile([128, 4 * L1], f32, name="cps1", tag="cps")
                        nc.tensor.matmul(
                            ps[:],
                            wa_t[:, ct * 128:(ct + 1) * 128],
                            xat[:, g * 4 * L1:(g + 1) * 4 * L1],
                            start=True, stop=True,
                        )
                        drain(y1t[ct][:, g * 4 * L1:(g + 1) * 4 * L1], ps[:],
                              bias_t[:, ct:ct + 1])

                    def emit_c2(g, ct):
                        # conv2: 3x2 accumulating matmuls per (group, co_t)
                        ps = cps.tile([128, 4 * L2], f32, name="cps2", tag="cps")
                        for k in range(3):
                            for ci in range(2):
                                j = k * 4 + ci * 2 + ct
                                nc.tensor.matmul(
                                    ps[:],
                                    w2_t[:, j * 128:(j + 1) * 128],
                                    y1v[ci][:, 4 * g:4 * g + 4, k:k + L2],
                                    start=(k == 0 and ci == 0),
                                    stop=(k == 2 and ci == 1),
                                )
                        drain(y2t[ct][:, g * 4 * L2:(g + 1) * 4 * L2], ps[:],
                              bias_t[:, 2 + ct:3 + ct])

                    def emit_c3(g, ct):
                        # conv3: l-major resident y3; the drain writes
                        # [l, 4 contiguous samples] runs (8B) while reading
                        # the s-major PSUM through a strided (l, s) view
                        s0 = c * G + 4 * g
                        ps = cps.tile([128, 4 * L3], f32, name="cps3", tag="cps")
                        for k in range(3):
                            for ci in range(2):
                                j = k * 4 + ci * 2 + ct
                                nc.tensor.matmul(
                                    ps[:],
                                    w3_t[:, j * 128:(j + 1) * 128],
                                    y2v[ci][:, 4 * g:4 * g + 4, k:k + L3],
                                    start=(k == 0 and ci == 0),
                                    stop=(k == 2 and ci == 1),
                                )
                        y3v = y3_t[ct][:].rearrange("p (l s) -> p l s", s=B_pc)
                        psv = ps[:].rearrange("p (s l) -> p l s", l=L3)
                        h = 66
                        nc.scalar.activation(y3v[:, 0:h, s0:s0 + 4],
                                             psv[:, 0:h, :], relu,
                                             bias=bias_t[:, 4 + ct:5 + ct])
                        nc.vector.tensor_scalar(y3v[:, h:L3, s0:s0 + 4],
                                                psv[:, h:L3, :],
                                                bias_t[:, 4 + ct:5 + ct],
                                                0.0, add_op, max_op)

                    for g in range(ngrp):
                        for ct in range(2):
                            emit_c1(g, ct)
                    for g in range(ngrp):
                        for ct in range(2):
                            emit_c2(g, ct)
                    for g in range(ngrp):
                        for ct in range(2):
                            emit_c3(g, ct)

            # ---- fc1: stream weights through the ring, accumulate in PSUM;
            # fc2 chained off each psf drain. A second ring pool reuses the
            # SBUF freed by the conv pools, deepening the stream buffer ----
            with (
                tc.tile_pool(name="wf1b", bufs=WF1B_RING) as wfp2,
                tc.tile_pool(name="fpsum", bufs=1, space="PSUM") as fps,
            ):
                psf = [fps.tile([128, B_pc], f32, name=f"psf_{i}", tag=f"psf_{i}") for i in range(NT)]
                for i in range(NL):
                    ct, l = divmod(i, L3)
                    if i < WF1_RING:
                        wt = wf_tiles[i]
                    else:
                        pool = wfp if i % 2 == 0 else wfp2
                        wt = pool.tile([128, F1], bf, name="wf1_t", tag="wf1")
                        dma_engs[i % 3].dma_start(out=wt[:], in_=wf1_d.ap()[i])
                    rhs = y3_t[ct][:, l * B_pc:(l + 1) * B_pc]
                    for nt in range(NT):
                        nc.tensor.matmul(
                            psf[nt],
                            wt[:, nt * 128:(nt + 1) * 128],
                            rhs,
                            start=(i == 0),
                            stop=(i == NL - 1),
                        )
                for bh in range(B_pc // 128):
                    for nt in range(NT):
                        drain(out1_t[:, nt * B_pc + bh * 128:
                                     nt * B_pc + (bh + 1) * 128],
                              psf[nt][:, bh * 128:(bh + 1) * 128],
                              bias_t[:, 6 + nt:7 + nt])

                # fc2: per-bh chains consume out1 blocks right behind the
                # psf drains; reuses drained psf banks (same pool, no pool
                # transition barrier); psum [samples, OUT] keeps the output
                # DMA contiguous
                for bh in range(B_pc // 128):
                    ps = fps.tile([128, OUT], f32, name=f"ops_t{bh}",
                                  tag=f"psf_{bh}")
                    for nt in range(NT):
                        nc.tensor.matmul(
                            ps[:],
                            out1_t[:, nt * B_pc + bh * 128:
                                   nt * B_pc + (bh + 1) * 128],
                            wf2_t[:, nt * OUT:(nt + 1) * OUT],
                            start=(nt == 0),
                            stop=(nt == NT - 1),
                        )
                    ot = osb_t[:, bh * OUT:(bh + 1) * OUT]
                    nc.vector.tensor_tensor(
                        out=ot, in0=ps[:],
                        in1=bias_t[:, 6 + NT:6 + NT + OUT],
                        op=add_op,
                    )
                    (nc.sync if bh == 0 else nc.scalar).dma_start(
                        out=out_d.ap()[bh * 128:(bh + 1) * 128, :],
                        in_=ot)

    nc.compile()
    return nc


def _host_prep(x, w1, b1, w2, b2, w3, b3, wfc1, bfc1, wfc2, bfc2, B_pc, G):
    """Build per-core input maps (shared weight arrays built once)."""
    NT = F1 // 128
    nchunks = B_pc // G

    # Augmented conv1 input: rows 0..63 = x0 broadcast, 64..127 = xr[:, :, 1:]
    B = x.shape[0]
    xr = np.ascontiguousarray(x.reshape(B, CL, IL).transpose(0, 2, 1))  # [B, IL, CL]
    xa = np.empty((B, 128, L1), dtype=np.float32)
    xa[:, :IL, :] = xr[:, :, 0:1]
    xa[:, IL:, :] = xr[:, :, 1:]
    xa = xa.astype(BF16)

    # conv1 weights: watilde[r, c] = w1[c, r, 0] (r<64) else w1[c, r-64, 1]
    wa = np.concatenate([w1[:, :, 0].T, w1[:, :, 1].T], axis=0).astype(BF16)
    wa = np.ascontiguousarray(wa)  # [128, 256]

    def conv_tiles(w):
        # w [co, ci, k] -> [ci(128), j*128+co], j = k*4 + ci_t*2 + co_t
        t = w.reshape(2, 128, 2, 128, 3)  # [co_t, co, ci_t, ci, k]
        t = t.transpose(4, 2, 0, 3, 1)    # [k, ci_t, co_t, ci, co]
        t = t.reshape(12, 128, 128).transpose(1, 0, 2).reshape(128, 12 * 128)
        return np.ascontiguousarray(t.astype(BF16))

    w2sb = conv_tiles(w2)
    w3sb = conv_tiles(w3)

    # fc1 weights: wf1[ct*123+l][co, nt*128+n] = wfc1[nt*128+n, (ct*128+co)*123+l]
    t = wfc1.reshape(F1, 2, 128, L3)      # [n, co_t, co, l]
    t = t.transpose(1, 3, 2, 0)           # [co_t, l, co, n]
    wf1 = np.ascontiguousarray(t.reshape(2 * L3, 128, F1).astype(BF16))

    # fc2: wf2[n, nt*16+o] = wfc2[o, nt*128+n]
    t = wfc2.T.reshape(NT, 128, OUT).transpose(1, 0, 2).reshape(128, NT * OUT)
    wf2 = np.ascontiguousarray(t.astype(BF16))

    bias = np.zeros((128, 6 + NT + OUT), dtype=np.float32)
    bias[:, 0:2] = b1.reshape(2, 128).T
    bias[:, 2:4] = b2.reshape(2, 128).T
    bias[:, 4:6] = b3.reshape(2, 128).T
    bias[:, 6:6 + NT] = bfc1.reshape(NT, 128).T
    bias[:, 6 + NT:] = bfc2[None, :]

    in_maps = []
    ncores = B // B_pc
    for ci in range(ncores):
        shard = xa[ci * B_pc:(ci + 1) * B_pc]            # [B_pc, 128, L1]
        shard = shard.reshape(nchunks, G, 128, L1).transpose(0, 2, 1, 3)
        shard = np.ascontiguousarray(shard).reshape(nchunks, 128, G * L1)
        in_maps.append({
            "xa": shard, "wa": wa, "w2": w2sb, "w3": w3sb,
            "wf1": wf1, "wf2": wf2, "bias": bias,
        })
    return in_maps


def kernel(x, w1, b1, w2, b2, w3, b3, wfc1, bfc1, wfc2, bfc2):
    B_pc = BATCH // NCORES
    G = 16
    key = ("prog", B_pc, G)
    if key not in _CACHE:
        _CACHE[key] = _build_program(B_pc, G)
    nc = _CACHE[key]
    in_maps = _host_prep(
        np.asarray(x, dtype=np.float32), np.asarray(w1, dtype=np.float32),
        np.asarray(b1, dtype=np.float32), np.asarray(w2, dtype=np.float32),
        np.asarray(b2, dtype=np.float32), np.asarray(w3, dtype=np.float32),
        np.asarray(b3, dtype=np.float32), np.asarray(wfc1, dtype=np.float32),
        np.asarray(bfc1, dtype=np.float32), np.asarray(wfc2, dtype=np.float32),
        np.asarray(bfc2, dtype=np.float32), B_pc, G,
    )
    trace = bool(os.environ.get("KERNEL_TRACE"))
    res = run_bass_kernel_spmd(nc, in_maps, core_ids=list(range(NCORES)),
                               trace=trace)
    _CACHE["last_results"] = res
    return np.concatenate([res.results[i]["out"] for i in range(NCORES)], axis=0)

